# revision 18
# baseline (speedup 1.0000x reference)
"""DNRI DynamicVars message-passing step as a Bass/Tile kernel on 8 TRN2 NeuronCores.

Algorithm (factored edge MLP over the dense N x N (send,recv) grid, recv-sharded):
  layer1 for edge (s -> v), type k:  m1 = tanh(A_k[v] + B_k[s])
      with A_k = hid @ W1r_k.T + b1_k  (receiver half of msg_w1)
           B_k = hid @ W1s_k.T         (sender half)
  layer2: m2 = tanh(m1 @ W2_k.T + b2_k)
  agg[v] = sum_k sum_s wgrid[k,s,v] * m2[k,v,s,:]
      wgrid = edge probabilities scattered to the grid / (K*(N-1)); zero where
      no edge (incl. the diagonal), so the dense grid reproduces the edge list.
  then GRU update + output MLP (replicated weights) in transposed [feat, node]
  layout on each core's 32 local nodes.

Each core owns 32 recv-nodes => aggregation is core-local, no collectives.

On-chip layouts: stage-1 tiles are [128 h-part, (v,s) free]; layer-2 matmuls
flip to [(v,s)-pair part, h2 free] so the edge weighting + segment-sum + k-sum
all become PSUM-accumulated M=1 matmuls with the weight column as lhsT.
"""
import sys
import numpy as np

sys.path.insert(0, "/opt/trn_rl_repo")

import concourse.bass as bass
import concourse.tile as tile
from concourse import bacc, mybir
from concourse.bass_utils import run_bass_kernel_spmd
from concourse.masks import make_identity

F32 = mybir.dt.float32
AF = mybir.ActivationFunctionType
ALU = mybir.AluOpType

N, H, D, K = 256, 256, 8, 4
NCORES = 8
VLOC = N // NCORES              # 32 recv nodes per core
VG = 8                          # v's per tanh1 super-tile (VG*N = 2048 free)
NVG = VLOC // VG                # 4 v-groups per core
Z2W = 1024                      # z2 psum tile free width (4 chunks of 128 pairs)
CPT = Z2W // N                  # chunks per z2 tile = 4
NHALF = VG * N // (128 * CPT)   # z2 tiles per (k, vg): 16 chunks / 4 = 4

_cache = {}


def _build_nc():
    nc = bacc.Bacc("TRN2", target_bir_lowering=False, debug=False,
                   num_devices=NCORES)

    # ---- kernel I/O ----------------------------------------------------
    d_hidT = nc.dram_tensor("hidT", [H, N], F32, kind="ExternalInput")
    d_w1rT = nc.dram_tensor("w1rT", [K, H, H], F32, kind="ExternalInput")
    d_w1sT = nc.dram_tensor("w1sT", [K, H, H], F32, kind="ExternalInput")
    d_b1 = nc.dram_tensor("b1c", [K, H, 1], F32, kind="ExternalInput")
    d_w2T = nc.dram_tensor("w2T", [K, H, H], F32, kind="ExternalInput")
    d_b2r = nc.dram_tensor("b2r", [1, K, N], F32, kind="ExternalInput")
    d_gx = nc.dram_tensor("gateX", [3, D + 1, H], F32, kind="ExternalInput")
    d_gh = nc.dram_tensor("gateH", [3, H, H], F32, kind="ExternalInput")
    d_o1 = nc.dram_tensor("o1T", [H, H], F32, kind="ExternalInput")
    d_o2 = nc.dram_tensor("o2T", [H, H], F32, kind="ExternalInput")
    d_o3 = nc.dram_tensor("o3T", [H, D], F32, kind="ExternalInput")
    d_ob = nc.dram_tensor("obias", [3, H, 1], F32, kind="ExternalInput")
    # per-core operands
    d_wg = nc.dram_tensor("aggW", [128, K, 2, VLOC, VLOC], F32,
                          kind="ExternalInput")
    d_xT9 = nc.dram_tensor("xT9", [D + 1, VLOC], F32, kind="ExternalInput")
    d_hidTl = nc.dram_tensor("hidTloc", [H, VLOC], F32, kind="ExternalInput")
    # outputs (transposed layouts, host re-transposes)
    d_predT = nc.dram_tensor("predT", [D, VLOC], F32, kind="ExternalOutput")
    d_newhT = nc.dram_tensor("newhT", [H, VLOC], F32, kind="ExternalOutput")

    with tile.TileContext(nc) as tc:
        with (
            tc.tile_pool(name="const", bufs=1) as constp,
            tc.tile_pool(name="wts", bufs=1) as wtp,
            tc.tile_pool(name="m1", bufs=2) as m1p,
            tc.tile_pool(name="t1", bufs=2) as t1p,
            tc.tile_pool(name="m2", bufs=3) as m2p,
            tc.tile_pool(name="epi", bufs=2) as epip,
            tc.tile_pool(name="z2", bufs=2, space=bass.MemorySpace.PSUM) as z2p,
            tc.tile_pool(name="aggp", bufs=1, space=bass.MemorySpace.PSUM) as aggpp,
            tc.tile_pool(name="eps", bufs=3, space=bass.MemorySpace.PSUM) as epipp,
        ):
            # ---- constants / weight staging -----------------------------
            ones_row = constp.tile([1, 128], F32)
            nc.gpsimd.memset(ones_row[:], 1.0)
            ident = constp.tile([VLOC, VLOC], F32)
            make_identity(nc, ident[:])

            hidT_s = wtp.tile([128, 2, N], F32, tag="hidT")
            nc.sync.dma_start(hidT_s[:], d_hidT[:].rearrange("(c p) n -> p c n", p=128))
            w1r_s = wtp.tile([128, K, 2, H], F32, tag="w1r")
            nc.sync.dma_start(w1r_s[:], d_w1rT[:].rearrange("k (c p) h -> p k c h", p=128))
            w1s_s = wtp.tile([128, K, 2, H], F32, tag="w1s")
            nc.sync.dma_start(w1s_s[:], d_w1sT[:].rearrange("k (c p) h -> p k c h", p=128))
            w2_s = wtp.tile([128, K, 2, H], F32, tag="w2")
            nc.sync.dma_start(w2_s[:], d_w2T[:].rearrange("k (c p) h -> p k c h", p=128))
            b1_s = wtp.tile([128, K, 2, 1], F32, tag="b1")
            nc.sync.dma_start(b1_s[:], d_b1[:].rearrange("k (c p) o -> p k c o", p=128))
            b2_s = wtp.tile([1, K, N], F32, tag="b2")
            nc.sync.dma_start(b2_s[:], d_b2r[:])
            wg_s = wtp.tile([128, K, 2, VLOC, VLOC], F32, tag="wg")
            nc.sync.dma_start(wg_s[:], d_wg[:])

            hidTl_s = wtp.tile([128, 2, VLOC], F32, tag="hidTl")
            nc.sync.dma_start(hidTl_s[:],
                              d_hidTl[:].rearrange("(c p) v -> p c v", p=128))

            # ---- A_k / B_k precompute ----------------------------------
            # A_kT[h, vloc] = sum_f W1r[k][h,f]*hid_local[v,f] + b1_k[h]
            # B_kT[h, s]    = sum_f W1s[k][h,f]*hid[s,f]        (all senders)
            A_s = wtp.tile([128, K, 2, VLOC], F32, tag="A")
            B_s = wtp.tile([128, K, 2, N], F32, tag="B")
            for k in range(K):
                for hc in range(2):
                    psa = epipp.tile([128, VLOC], F32, tag="eps")
                    for fc in range(2):
                        nc.tensor.matmul(
                            psa[:], w1r_s[:, k, fc, hc * 128:(hc + 1) * 128],
                            hidTl_s[:, fc, :], start=(fc == 0), stop=(fc == 1))
                    nc.scalar.activation(A_s[:, k, hc, :], psa[:],
                                         AF.Identity, bias=b1_s[:, k, hc, :])
                    psb = epipp.tile([128, N], F32, tag="eps")
                    for fc in range(2):
                        nc.tensor.matmul(
                            psb[:], w1s_s[:, k, fc, hc * 128:(hc + 1) * 128],
                            hidT_s[:, fc, :], start=(fc == 0), stop=(fc == 1))
                    nc.vector.tensor_copy(B_s[:, k, hc, :], psb[:])

            # ---- per-node aggregate accumulator ------------------------
            agg_ps = aggpp.tile([VLOC, H], F32)

            # ---- main loop over edge types and v-groups ----------------
            for k in range(K):
                for vg in range(NVG):
                    t1 = []
                    for hc in range(2):
                        m1 = m1p.tile([128, VG * N], F32, tag="m1")
                        for j in range(VG):
                            v = vg * VG + j
                            nc.vector.tensor_scalar_add(
                                m1[:, j * N:(j + 1) * N],
                                B_s[:, k, hc, :],               # B_kT[h, s-all]
                                A_s[:, k, hc, v:v + 1],         # A_kT[h, vloc]
                            )
                        t = t1p.tile([128, VG * N], F32, tag=f"t1_{hc}")
                        nc.scalar.activation(t[:], m1[:], AF.Tanh)
                        t1.append(t)
                    for half in range(NHALF):
                        z2 = z2p.tile([128, Z2W], F32, tag="z2")
                        for c in range(CPT):
                            p_idx = half * CPT + c
                            e0 = p_idx * 128
                            reg = z2[:, c * N:(c + 1) * N]
                            # init region with b2 broadcast (K=1 matmul), then
                            # accumulate the two h-chunk contractions
                            nc.tensor.matmul(reg, ones_row[:],
                                             b2_s[:, k, :],
                                             start=True, stop=False)
                            for hc in range(2):
                                nc.tensor.matmul(
                                    reg, t1[hc][:, e0:e0 + 128],
                                    w2_s[:, k, hc, :],
                                    start=False, stop=(hc == 1),
                                )
                        m2 = m2p.tile([128, Z2W], F32, tag="m2")
                        nc.scalar.activation(m2[:], z2[:], AF.Tanh)
                        # weighting + segment-sum + k-sum: accumulating MMs
                        # with a one-hot-column weight matrix as lhsT
                        for c in range(CPT):
                            p_idx = half * CPT + c
                            j, sc = p_idx >> 1, p_idx & 1
                            v = vg * VG + j
                            nc.tensor.matmul(
                                agg_ps[:],
                                wg_s[:, k, sc, v, :],
                                m2[:, c * N:(c + 1) * N],
                                start=(k == 0 and vg == 0 and p_idx == 0),
                                stop=(k == K - 1 and vg == NVG - 1
                                      and p_idx == 2 * VG - 1),
                            )

            # ---- epilogue: GRU + output MLP on the 32 local nodes ------
            gx_s = wtp.tile([D + 1, 3, H], F32, tag="gx")
            nc.sync.dma_start(gx_s[:], d_gx[:].rearrange("g d h -> d g h"))
            gh_s = wtp.tile([128, 3, 2, H], F32, tag="gh")
            nc.sync.dma_start(gh_s[:], d_gh[:].rearrange("g (c p) h -> p g c h", p=128))
            o1_s = wtp.tile([128, 2, H], F32, tag="o1")
            nc.sync.dma_start(o1_s[:], d_o1[:].rearrange("(c p) h -> p c h", p=128))
            o2_s = wtp.tile([128, 2, H], F32, tag="o2")
            nc.sync.dma_start(o2_s[:], d_o2[:].rearrange("(c p) h -> p c h", p=128))
            o3_s = wtp.tile([128, 2, D], F32, tag="o3")
            nc.sync.dma_start(o3_s[:], d_o3[:].rearrange("(c p) d -> p c d", p=128))
            ob_s = wtp.tile([128, 3, 2, 1], F32, tag="ob")
            nc.sync.dma_start(ob_s[:], d_ob[:].rearrange("g (c p) o -> p g c o", p=128))
            xT9_s = wtp.tile([D + 1, VLOC], F32, tag="xT9")
            nc.sync.dma_start(xT9_s[:], d_xT9[:])

            # agg [32, 256] -> sbuf -> transpose to aggT [128, hc, 32]
            agg_sb = epip.tile([VLOC, H], F32, tag="aggsb")
            nc.vector.tensor_copy(agg_sb[:], agg_ps[:])
            aggT = epip.tile([128, 2, VLOC], F32, tag="aggT")
            for hc in range(2):
                tp = epipp.tile([128, VLOC], F32, tag="eps")
                nc.tensor.transpose(tp[:], agg_sb[:, hc * 128:(hc + 1) * 128],
                                    ident[:])
                nc.vector.tensor_copy(aggT[:, hc, :], tp[:])

            # gates r (g=0) and i (g=1): sigmoid(x) = 0.5*tanh(0.5 x)+0.5
            r_sb = epip.tile([128, 2, VLOC], F32, tag="r")
            i_sb = epip.tile([128, 2, VLOC], F32, tag="i")
            for g, dst in ((0, r_sb), (1, i_sb)):
                for hc in range(2):
                    ps = epipp.tile([128, VLOC], F32, tag="eps")
                    nc.tensor.matmul(ps[:], gx_s[:, g, hc * 128:(hc + 1) * 128],
                                     xT9_s[:], start=True, stop=False)
                    for cc in range(2):
                        nc.tensor.matmul(
                            ps[:], gh_s[:, g, cc, hc * 128:(hc + 1) * 128],
                            aggT[:, cc, :], start=False, stop=(cc == 1))
                    th = epip.tile([128, VLOC], F32, tag="sig")
                    nc.scalar.activation(th[:], ps[:], AF.Tanh, scale=0.5)
                    nc.vector.tensor_scalar(dst[:, hc, :], th[:], 0.5, 0.5,
                                            ALU.mult, ALU.add)
            # n = tanh(xpart + r * aggpart)
            n_sb = epip.tile([128, 2, VLOC], F32, tag="n")
            for hc in range(2):
                psx = epipp.tile([128, VLOC], F32, tag="eps")
                nc.tensor.matmul(psx[:], gx_s[:, 2, hc * 128:(hc + 1) * 128],
                                 xT9_s[:], start=True, stop=True)
                psh = epipp.tile([128, VLOC], F32, tag="eps")
                for cc in range(2):
                    nc.tensor.matmul(psh[:], gh_s[:, 2, cc, hc * 128:(hc + 1) * 128],
                                     aggT[:, cc, :], start=(cc == 0), stop=(cc == 1))
                tmp = epip.tile([128, VLOC], F32, tag="ntmp")
                nc.vector.tensor_mul(tmp[:], r_sb[:, hc, :], psh[:])
                tmp2 = epip.tile([128, VLOC], F32, tag="ntmp2")
                nc.vector.tensor_add(tmp2[:], tmp[:], psx[:])
                nc.scalar.activation(n_sb[:, hc, :], tmp2[:], AF.Tanh)
            # new_h = n + i*(hid - n)
            newh = epip.tile([128, 2, VLOC], F32, tag="newh")
            for hc in range(2):
                dtmp = epip.tile([128, VLOC], F32, tag="dtmp")
                nc.vector.tensor_sub(dtmp[:], hidTl_s[:, hc, :], n_sb[:, hc, :])
                dtmp2 = epip.tile([128, VLOC], F32, tag="dtmp2")
                nc.vector.tensor_mul(dtmp2[:], i_sb[:, hc, :], dtmp[:])
                nc.vector.tensor_add(newh[:, hc, :], n_sb[:, hc, :], dtmp2[:])
                nc.sync.dma_start(
                    d_newhT[:].rearrange("(c p) v -> c p v", p=128)[hc],
                    newh[:, hc, :])

            # output MLP
            def mlp_layer(src, wT, gidx):
                dst = epip.tile([128, 2, VLOC], F32, tag=f"mlp{gidx}")
                for hc in range(2):
                    ps = epipp.tile([128, VLOC], F32, tag="eps")
                    for cc in range(2):
                        nc.tensor.matmul(ps[:], wT[:, cc, hc * 128:(hc + 1) * 128],
                                         src[:, cc, :], start=(cc == 0),
                                         stop=(cc == 1))
                    nc.scalar.activation(dst[:, hc, :], ps[:], AF.Relu,
                                         bias=ob_s[:, gidx, hc, :])
                return dst

            p1 = mlp_layer(newh, o1_s, 0)
            p2 = mlp_layer(p1, o2_s, 1)
            ps3 = epipp.tile([D, VLOC], F32, tag="eps")
            for cc in range(2):
                nc.tensor.matmul(ps3[:], o3_s[:, cc, :], p2[:, cc, :],
                                 start=(cc == 0), stop=(cc == 1))
            predT = epip.tile([D, VLOC], F32, tag="predT")
            # pred = (p3 + b_o3) + x
            nc.vector.scalar_tensor_tensor(predT[:], ps3[:], ob_s[0:D, 2, 0, :],
                                           xT9_s[0:D, :], ALU.add, ALU.add)
            nc.sync.dma_start(d_predT[:], predT[:])

    nc.compile()
    return nc


def _prep(inputs):
    """Host-side repack of the full inputs into per-core in_maps."""
    hid = np.ascontiguousarray(np.asarray(inputs["hidden"], np.float32)[0])
    x = np.ascontiguousarray(np.asarray(inputs["inputs"], np.float32)[0])
    edges = np.asarray(inputs["edges"], np.float32)[0]                # [E, K]
    send = np.asarray(inputs["send_edges"]).astype(np.int64)
    recv = np.asarray(inputs["recv_edges"]).astype(np.int64)
    w1 = np.asarray(inputs["msg_w1"], np.float32)
    w2 = np.asarray(inputs["msg_w2"], np.float32)

    wg = np.zeros((K, N, N), np.float32)                              # [k, s, v]
    wg[:, send, recv] = edges.T
    wg *= 1.0 / (K * (N - 1))

    gx = np.stack([
        np.concatenate([np.asarray(inputs[f"w_{g}"], np.float32).T,
                        np.asarray(inputs[f"b_{g}"], np.float32)[None, :]], 0)
        for g in ("ir", "ii", "in")])                                 # [3, 9, H]
    gh = np.stack([np.ascontiguousarray(np.asarray(inputs[f"w_h{g}"],
                                                   np.float32).T)
                   for g in ("r", "i", "h")])                         # [3, H, H]
    ob = np.stack([np.pad(np.asarray(inputs[f"b_o{j}"], np.float32),
                          (0, H - (D if j == 3 else H)))[:, None]
                   for j in (1, 2, 3)])                               # [3, H, 1]

    rep = {
        "hidT": np.ascontiguousarray(hid.T),
        "w1rT": np.ascontiguousarray(np.swapaxes(w1[:, :, :H], 1, 2)),
        "w1sT": np.ascontiguousarray(np.swapaxes(w1[:, :, H:], 1, 2)),
        "b1c": np.ascontiguousarray(
            np.asarray(inputs["msg_b1"], np.float32)[:, :, None]),
        "w2T": np.ascontiguousarray(np.swapaxes(w2, 1, 2)),
        "b2r": np.ascontiguousarray(
            np.asarray(inputs["msg_b2"], np.float32)[None]),
        "gateX": np.ascontiguousarray(gx),
        "gateH": np.ascontiguousarray(gh),
        "o1T": np.ascontiguousarray(np.asarray(inputs["w_o1"], np.float32).T),
        "o2T": np.ascontiguousarray(np.asarray(inputs["w_o2"], np.float32).T),
        "o3T": np.ascontiguousarray(np.asarray(inputs["w_o3"], np.float32).T),
        "obias": np.ascontiguousarray(ob),
    }

    in_maps = []
    for c in range(NCORES):
        v0 = c * VLOC
        # aggW[p, k, sc, v, col] = wg[k, sc*128+p, v0+v] * (col == v)
        aggw = np.zeros((128, K, 2, VLOC, VLOC), np.float32)
        blk = wg[:, :, v0:v0 + VLOC].reshape(K, 2, 128, VLOC)  # [k, sc, p, v]
        vv = np.arange(VLOC)
        aggw[:, :, :, vv, vv] = np.moveaxis(blk, 2, 0)
        xT9 = np.concatenate([x[v0:v0 + VLOC].T,
                              np.ones((1, VLOC), np.float32)], 0)
        m = dict(rep)
        m["aggW"] = np.ascontiguousarray(aggw)
        m["xT9"] = np.ascontiguousarray(xT9)
        m["hidTloc"] = np.ascontiguousarray(hid[v0:v0 + VLOC].T)
        in_maps.append(m)
    return in_maps


def kernel(**inputs):
    if "nc" not in _cache:
        _cache["nc"] = _build_nc()
    nc = _cache["nc"]
    in_maps = _prep(inputs)
    res = run_bass_kernel_spmd(nc, in_maps, core_ids=list(range(NCORES)))
    _cache["last_results"] = res
    pred = np.zeros((1, N, D), np.float32)
    hout = np.zeros((1, N, H), np.float32)
    for c in range(NCORES):
        v0 = c * VLOC
        pred[0, v0:v0 + VLOC] = res.results[c]["predT"].T
        hout[0, v0:v0 + VLOC] = res.results[c]["newhT"].T
    return pred, hout
